# revision 1
# baseline (speedup 1.0000x reference)
"""Trainium2 Bass kernel for nn_LitToClauseLayer (gather + segment_sum + LSTM cell).

Reference computation:
    msg   = segment_sum(x_l[edge_lit], edge_clause, num_segments=N_CLAUSE)   # [NC, D]
    gates = msg @ W_ih.T + b_ih + h0 @ W_hh.T + b_hh                         # [NC, 4D]
    i, f, g, o = split(gates); i,f,o = sigmoid; g = tanh
    c_new = f*c0 + i*g ; h_new = o*tanh(c_new)
    returns (h_new, c_new)

Distribution (8 cores, SPMD):
  - Clauses (and the clause-sorted edge list) are sharded across 8 cores;
    x_l (bf16 copy) and the LSTM weights are replicated.
  - Everything on-device is kept feature-major ([D=128 partitions, clause])
    so no transposes are needed and the ACT engine's per-partition bias
    implements the gate biases exactly.
  - Edge gathering uses the ant dma_gather ucode (int16 indices -> x_l is
    split into 4 row-banks of 25000). Edges are grouped per 128-clause chunk,
    sorted by bank within the chunk, and each (chunk, bank) cell is padded to
    a multiple of 128 slots (budget shared across cores so the program is
    SPMD-identical).  One dma_gather per (4-chunk superchunk, bank), spread
    over 4 SWDGE queues for parallel descriptor generation.
  - Segment sum per 128-edge tile t (single chunk by construction):
        msgT[D, C=128] += xg_t[e, D].T @ onehot_t[e, C]
    with onehot built on DVE: is_equal(iota[p,c]=c, eloc[p,t] broadcast).
  - Gates per 128-block g: gT_g[128, C] = W_ih_g @ msgT + W_hh_g @ h0T in
    PSUM; ACT applies sigmoid/tanh with bias = (b_ih+b_hh) slice; pointwise
    LSTM update on DVE; outputs stored transposed, un-transposed on host.
"""

import numpy as np

N_LIT, N_CLAUSE, N_EDGES, D = 100000, 400000, 1200000, 128
N_CORES = 8
CPC = N_CLAUSE // N_CORES  # clauses per core (50000)
P = 128
BANK_ROWS = 25000
SC_CHUNKS = 4              # chunks per superchunk
MAX_CALL_IDX = 1024        # dma_gather crashes above ~1024 idxs/call

_cache = {}

# test-harness hooks (ignored in normal use)
TRACE = False
LAST_RESULT = None


def _compute_structure(edge_lit, edge_clause, n_lit=N_LIT, cpc=CPC,
                       bank_rows=BANK_ROWS, sc_chunks=SC_CHUNKS,
                       n_cores=N_CORES):
    """Shared (SPMD) program structure + per-core edge placement.

    Returns (structure, per_core) where structure determines the Bass program
    and per_core[k] carries the cell edge arrays for table building.
    """
    n_banks = -(-n_lit // bank_rows)
    n_chunks = -(-cpc // P)
    n_chunks = -(-n_chunks // sc_chunks) * sc_chunks
    n_sc = n_chunks // sc_chunks

    counts = np.zeros((n_cores, n_chunks, n_banks), dtype=np.int64)
    per_core = []
    for k in range(n_cores):
        marks = k * cpc + P * np.arange(n_chunks + 1)
        marks = np.minimum(marks, (k + 1) * cpc)
        bounds = np.searchsorted(edge_clause, marks, side="left")
        cells = {}
        for j in range(n_chunks):
            e0, e1 = bounds[j], bounds[j + 1]
            if e1 > e0:
                lits = edge_lit[e0:e1]
                locs = (edge_clause[e0:e1] - (k * cpc + j * P)).astype(np.int16)
                banks = lits // bank_rows
                order = np.argsort(banks, kind="stable")
                lits, locs, banks = lits[order], locs[order], banks[order]
                for b in range(n_banks):
                    m = banks == b
                    if m.any():
                        cells[(j, b)] = (lits[m] - b * bank_rows, locs[m])
                        counts[k, j, b] = int(m.sum())
        per_core.append(cells)

    maxc = counts.max(axis=0)                       # [n_chunks, n_banks]
    budgets = ((maxc + P - 1) // P) * P             # slots per cell
    for j in range(n_chunks):                       # every chunk >= 1 tile
        if budgets[j].sum() == 0:
            budgets[j, 0] = P

    structure = {
        "n_lit": n_lit, "n_banks": n_banks, "bank_rows": bank_rows,
        "n_chunks": n_chunks, "n_sc": n_sc, "sc_chunks": sc_chunks,
        "budgets": budgets,
    }
    # derived call geometry
    call_nidx = np.zeros((n_sc, n_banks), dtype=np.int64)
    for s in range(n_sc):
        for b in range(n_banks):
            call_nidx[s, b] = budgets[s * sc_chunks:(s + 1) * sc_chunks, b].sum()
    assert call_nidx.max() <= MAX_CALL_IDX, (
        f"gather call too large: {call_nidx.max()}")
    structure["call_nidx"] = call_nidx
    return structure, per_core


def _build_program(st):
    import concourse.bacc as bacc
    import concourse.bass as bass
    import concourse.mybir as mybir
    import concourse.tile as tile

    dt = mybir.dt
    n_chunks, n_sc, scc = st["n_chunks"], st["n_sc"], st["sc_chunks"]
    n_banks, bank_rows = st["n_banks"], st["bank_rows"]
    budgets, call_nidx = st["budgets"], st["call_nidx"]
    n_lit = st["n_lit"]
    ncols = n_chunks * P
    sc_cols = scc * P
    total_idx_cols = int(call_nidx.sum()) // 16
    total_tiles = int(budgets.sum()) // P

    nc = bacc.Bacc(None, target_bir_lowering=False, num_swdge_queues=4)

    xlb = nc.dram_tensor("xlb", [n_lit, D], dt.float16, kind="ExternalInput")
    h0t = nc.dram_tensor("h0t", [P, ncols], dt.float16, kind="ExternalInput")
    c0t = nc.dram_tensor("c0t", [P, ncols], dt.float16, kind="ExternalInput")
    eidx = nc.dram_tensor("eidx", [P, total_idx_cols], dt.int16, kind="ExternalInput")
    eloc = nc.dram_tensor("eloc", [P, total_tiles], dt.float16, kind="ExternalInput")
    wih = nc.dram_tensor("wih", [P, 4 * D], dt.float16, kind="ExternalInput")
    whh = nc.dram_tensor("whh", [P, 4 * D], dt.float16, kind="ExternalInput")
    btab = nc.dram_tensor("btab", [P, 4], dt.float32, kind="ExternalInput")
    iota = nc.dram_tensor("iota", [P, P], dt.float16, kind="ExternalInput")
    ht = nc.dram_tensor("ht", [P, ncols], dt.float16, kind="ExternalOutput")
    ct = nc.dram_tensor("ct", [P, ncols], dt.float16, kind="ExternalOutput")

    f32, bf16, fp16, i16 = dt.float32, dt.bfloat16, dt.float16, dt.int16
    Sig = mybir.ActivationFunctionType.Sigmoid
    Tnh = mybir.ActivationFunctionType.Tanh

    # per-sc geometry (compile-time)
    max_nt = 0
    max_bank_nt = [0] * n_banks
    sc_geom = []
    icol = 0
    tcol = 0
    for s in range(n_sc):
        calls = []       # per bank: (icol_base, nidx, ntile)
        tiles = []       # bank-major: (bank, tile_in_call, chunk_local)
        for b in range(n_banks):
            nidx = int(call_nidx[s, b])
            t_in_call = 0
            for c in range(scc):
                j = s * scc + c
                for _ in range(int(budgets[j, b]) // P):
                    tiles.append((b, t_in_call, c))
                    t_in_call += 1
            calls.append((icol, nidx, t_in_call))
            icol += nidx // 16
            max_bank_nt[b] = max(max_bank_nt[b], t_in_call)
        sc_geom.append((calls, tiles, tcol))
        tcol += len(tiles)
        max_nt = max(max_nt, len(tiles))

    with tile.TileContext(nc) as tc:
        with (
            tc.tile_pool(name="const", bufs=1) as cpool,
            tc.tile_pool(name="tabs", bufs=3) as tabs,
            tc.tile_pool(name="gat", bufs=3) as gat,
            tc.tile_pool(name="oh", bufs=3) as ohp,
            tc.tile_pool(name="hc", bufs=3) as hcp,
            tc.tile_pool(name="msg", bufs=4) as msgp,
            tc.tile_pool(name="acts", bufs=4) as actp,
            tc.tile_pool(name="outs", bufs=3) as outp,
            tc.tile_pool(name="pm", bufs=2, space="PSUM") as pmsum,
            tc.tile_pool(name="pg", bufs=1, space="PSUM") as pgate,
        ):
            wih_sb = cpool.tile([P, 4 * D], fp16, tag="wih")
            whh_sb = cpool.tile([P, 4 * D], fp16, tag="whh")
            b_sb = cpool.tile([P, 4], f32, tag="btab")
            iota_sb = cpool.tile([P, P], fp16, tag="iota")
            nc.sync.dma_start(out=wih_sb[:], in_=wih[:, :])
            nc.sync.dma_start(out=whh_sb[:], in_=whh[:, :])
            nc.sync.dma_start(out=b_sb[:], in_=btab[:, :])
            nc.sync.dma_start(out=iota_sb[:], in_=iota[:, :])

            for s in range(n_sc):
                calls, tiles, tbase = sc_geom[s]
                n_t = len(tiles)
                sc_icol = calls[0][0]
                sc_icols = sum(nidx // 16 for _, nidx, _ in calls)

                idx_t = tabs.tile([P, max(sc_icols, 16)], i16, tag="idx")
                nc.sync.dma_start(out=idx_t[:, :sc_icols],
                                  in_=eidx[:, sc_icol:sc_icol + sc_icols])
                elo_t = tabs.tile([P, max_nt], fp16, tag="elo")
                nc.sync.dma_start(out=elo_t[:, :n_t],
                                  in_=eloc[:, tbase:tbase + n_t])

                g_tiles = []
                for b in range(n_banks):
                    icol0, nidx, ntile = calls[b]
                    if nidx == 0:
                        g_tiles.append(None)
                        continue
                    g_t = gat.tile([P, max_bank_nt[b], D], fp16, tag=f"g{b}")
                    lo = b * bank_rows
                    hi = min(lo + bank_rows, n_lit)
                    nc.gpsimd.dma_gather(
                        out_ap=g_t[:, :ntile, :],
                        in_ap=xlb[lo:hi, :],
                        idxs_ap=idx_t[:, icol0 - sc_icol:
                                      icol0 - sc_icol + nidx // 16],
                        num_idxs=nidx, num_idxs_reg=nidx, elem_size=D,
                        queue_num=b % 4)
                    g_tiles.append(g_t)

                oh_t = ohp.tile([P, max_nt, P], fp16, tag="onehot")
                i_ap = iota_sb[:]
                iota_b = bass.AP(i_ap.tensor, i_ap.offset,
                                 [i_ap.ap[0], [0, n_t], [1, P]])
                e_ap = elo_t[:]
                elo_b = bass.AP(e_ap.tensor, e_ap.offset,
                                [e_ap.ap[0], [1, n_t], [0, P]])
                nc.vector.tensor_tensor(out=oh_t[:, :n_t, :], in0=iota_b,
                                        in1=elo_b, op=mybir.AluOpType.is_equal)

                h0_t = hcp.tile([P, sc_cols], fp16, tag="h0")
                c0_t = hcp.tile([P, sc_cols], fp16, tag="c0")
                nc.sync.dma_start(out=h0_t[:], in_=h0t[:, s * sc_cols:(s + 1) * sc_cols])
                nc.sync.dma_start(out=c0_t[:], in_=c0t[:, s * sc_cols:(s + 1) * sc_cols])

                ht_acc = outp.tile([P, sc_cols], fp16, tag="htacc")
                ct_acc = outp.tile([P, sc_cols], fp16, tag="ctacc")

                # group tiles by chunk
                by_chunk = [[] for _ in range(scc)]
                for ti, (b, t, c) in enumerate(tiles):
                    by_chunk[c].append((b, t, ti))

                # segment sums for all chunks of the sc -> one PSUM bank
                msg_ps = pmsum.tile([P, scc * P], f32, tag="msgps")
                for c in range(scc):
                    lst = by_chunk[c]
                    for k, (b, t, ti) in enumerate(lst):
                        nc.tensor.matmul(
                            out=msg_ps[:, c * P:(c + 1) * P],
                            lhsT=g_tiles[b][:, t, :],
                            rhs=oh_t[:, ti, :],
                            start=(k == 0),
                            stop=(k == len(lst) - 1),
                        )
                msg_sb = msgp.tile([P, scc * P], fp16, tag="msgsb")
                nc.vector.tensor_copy(out=msg_sb[:], in_=msg_ps[:])

                # gates for the whole sc: [P, scc*512] PSUM (scc banks)
                gate_ps = pgate.tile([P, scc * 4 * D], f32, tag="gateps")
                for c in range(scc):
                    for g in range(4):
                        gs = slice(c * 4 * D + g * D, c * 4 * D + (g + 1) * D)
                        nc.tensor.matmul(out=gate_ps[:, gs], lhsT=wih_sb[:, g * D:(g + 1) * D],
                                         rhs=msg_sb[:, c * P:(c + 1) * P], start=True, stop=False)
                        nc.tensor.matmul(out=gate_ps[:, gs], lhsT=whh_sb[:, g * D:(g + 1) * D],
                                         rhs=h0_t[:, c * P:(c + 1) * P], start=False, stop=True)

                # batched activations: gate g across chunks = strided AP
                gp = gate_ps[:]
                part0 = gp.ap[0]

                def gate_view(g):
                    return bass.AP(gp.tensor, gp.offset + g * D,
                                   [part0, [4 * D, scc], [1, D]])

                i_s = actp.tile([P, scc, P], fp16, tag="i_s")
                f_s = actp.tile([P, scc, P], fp16, tag="f_s")
                g_s = actp.tile([P, scc, P], fp16, tag="g_s")
                o_s = actp.tile([P, scc, P], fp16, tag="o_s")
                nc.scalar.activation(i_s[:], gate_view(0), Sig, bias=b_sb[:, 0:1])
                nc.scalar.activation(f_s[:], gate_view(1), Sig, bias=b_sb[:, 1:2])
                nc.scalar.activation(o_s[:], gate_view(3), Sig, bias=b_sb[:, 3:4])
                nc.scalar.activation(g_s[:], gate_view(2), Tnh, bias=b_sb[:, 2:3])

                def flat(t):
                    a = t[:]
                    return bass.AP(a.tensor, a.offset, [a.ap[0], [1, scc * P]])

                t1 = actp.tile([P, scc * P], fp16, tag="t1")
                t2 = actp.tile([P, scc * P], fp16, tag="t2")
                nc.vector.tensor_mul(out=t1[:], in0=flat(f_s), in1=c0_t[:])
                nc.vector.tensor_mul(out=t2[:], in0=flat(i_s), in1=flat(g_s))
                nc.vector.tensor_add(out=ct_acc[:], in0=t1[:], in1=t2[:])
                tnh_c = actp.tile([P, scc * P], fp16, tag="tnhc")
                nc.scalar.activation(tnh_c[:], ct_acc[:], Tnh)
                nc.vector.tensor_mul(out=ht_acc[:], in0=flat(o_s), in1=tnh_c[:])

                nc.sync.dma_start(out=ht[:, s * sc_cols:(s + 1) * sc_cols], in_=ht_acc[:])
                nc.sync.dma_start(out=ct[:, s * sc_cols:(s + 1) * sc_cols], in_=ct_acc[:])

    nc.compile()
    return nc


def _prep_core_inputs(core, inputs, st, cells):
    import ml_dtypes

    x_l = inputs["x_l"]
    h0, c0 = inputs["h0"], inputs["c0"]
    n_chunks, n_sc, scc = st["n_chunks"], st["n_sc"], st["sc_chunks"]
    n_banks = st["n_banks"]
    budgets, call_nidx = st["budgets"], st["call_nidx"]
    ncols = n_chunks * P
    cpc = CPC
    c_lo = core * cpc

    h0t = np.zeros((P, ncols), dtype=np.float16)
    c0t = np.zeros((P, ncols), dtype=np.float16)
    h0t[:, :cpc] = h0[c_lo:c_lo + cpc].T.astype(np.float16)
    c0t[:, :cpc] = c0[c_lo:c_lo + cpc].T.astype(np.float16)

    total_idx_cols = int(call_nidx.sum()) // 16
    total_tiles = int(budgets.sum()) // P
    eidx = np.zeros((P, total_idx_cols), dtype=np.int16)
    eloc_flat = np.full(total_tiles * P, -1.0, dtype=np.float16)

    icol = 0
    slot0 = 0
    for s in range(n_sc):
        for b in range(n_banks):
            nidx = int(call_nidx[s, b])
            if nidx == 0:
                continue
            flat = (np.arange(nidx, dtype=np.int64) * 97
                    % min(st["bank_rows"], st["n_lit"] - b * st["bank_rows"])
                    ).astype(np.int16)
            off = 0
            for c in range(scc):
                j = s * scc + c
                bud = int(budgets[j, b])
                if bud == 0:
                    continue
                lits, locs = cells.get((j, b), (None, None))
                if lits is not None:
                    n = len(lits)
                    flat[off:off + n] = lits
                    eloc_flat[slot0 + off:slot0 + off + n] = locs
                off += bud
            blk = flat.reshape(nidx // 16, 16).T
            for r in range(8):
                eidx[16 * r:16 * (r + 1), icol:icol + nidx // 16] = blk
            icol += nidx // 16
            slot0 += nidx
    # eloc layout [p, tile] with slot = tile*128 + p
    eloc = eloc_flat.reshape(total_tiles, P).T.copy()

    return {"xlb": np.ascontiguousarray(x_l.astype(np.float16)),
            "h0t": h0t, "c0t": c0t, "eidx": eidx, "eloc": eloc}


def _shared_inputs(inputs):
    import ml_dtypes

    W_ih, W_hh = inputs["W_ih"], inputs["W_hh"]
    b2 = (inputs["b_ih"] + inputs["b_hh"]).astype(np.float32)
    wih = np.ascontiguousarray(W_ih.T.astype(np.float16))
    whh = np.ascontiguousarray(W_hh.T.astype(np.float16))
    btab = np.ascontiguousarray(b2.reshape(4, P).T)
    iota = np.broadcast_to(np.arange(P, dtype=np.float16), (P, P))
    return {"wih": wih, "whh": whh, "btab": btab,
            "iota": np.ascontiguousarray(iota)}


def kernel(x_l, h0, c0, W_ih, W_hh, b_ih, b_hh, edge_lit, edge_clause):
    from concourse.bass_utils import run_bass_kernel_spmd

    inputs = dict(x_l=x_l, h0=h0, c0=c0, W_ih=W_ih, W_hh=W_hh, b_ih=b_ih,
                  b_hh=b_hh, edge_lit=edge_lit, edge_clause=edge_clause)

    st, per_core = _compute_structure(np.asarray(edge_lit),
                                      np.asarray(edge_clause))
    key = ("v4a", st["n_chunks"], st["n_banks"], st["budgets"].tobytes())
    if key not in _cache:
        _cache[key] = _build_program(st)
    nc = _cache[key]

    shared = _shared_inputs(inputs)
    in_maps = []
    for k in range(N_CORES):
        m = _prep_core_inputs(k, inputs, st, per_core[k])
        m.update(shared)
        in_maps.append(m)

    res = run_bass_kernel_spmd(nc, in_maps, core_ids=list(range(N_CORES)),
                               trace=TRACE)
    global LAST_RESULT
    LAST_RESULT = res

    h_new = np.empty((N_CLAUSE, D), dtype=np.float32)
    c_new = np.empty((N_CLAUSE, D), dtype=np.float32)
    for k in range(N_CORES):
        out = res.results[k]
        h_new[k * CPC:(k + 1) * CPC] = out["ht"][:, :CPC].T.astype(np.float32)
        c_new[k * CPC:(k + 1) * CPC] = out["ct"][:, :CPC].T.astype(np.float32)
    return (h_new, c_new)

